# revision 24
# baseline (speedup 1.0000x reference)
"""Trainium2 Bass kernel for linear multi-head attention.

Reference computation (B=4, S=8192, D=1024, H=16, DH=64):
    Q  = softmax((x@Wq) per-head over DH) * DH**-0.5
    K  = softmax((x@Wkv)[...,:DH] per-head over S)
    V  = (x@Wkv)[..., DH:]
    ctx = K^T @ V  per (b, h)               # [DH, DH]
    y  = (Q @ ctx  per head) @ Wlin + blin

Sharding: sequence-parallel over 8 NeuronCores. Each core handles
S_LOC = 1024 rows per batch element. The K-softmax runs over the full
sequence, so each core accumulates unnormalized per-(b,h) context
ctxU = sum_s exp(k_s) v_s and Z = sum_s exp(k_s) locally, and a single
AllReduce sums them across cores. (No max-subtraction is needed:
|k| <= ~4 for these input statistics, exp stays in fp32 range.)

Per-core pipeline (all matmuls contract over the partition dim):
  pass 1: x tile -> PE-transpose -> KV = x@Wkv (fp32r, full rate)
          -> exp(K) (bf16) -> per-head ctxU/Z accumulation in PSUM
  allreduce [128, B*8*65] fp32 (ctxU + Z packed)
  mid:    ctxn = ctxU/Z * SCALE -> PE-transpose -> M_h = ctxn_h @ Wlin_h
          packed to M [D, D] per batch (bf16)  (y = Qn @ M + blin)
  pass 2: x tile -> PE-transpose -> Q = x@Wq (fp32r) -> exp
          -> row-normalize (free-dim reduce) -> PE-transpose (bf16)
          -> y = Qn @ M (bf16) + blin -> store
"""

import sys

if "/opt/trn_rl_repo" not in sys.path:
    sys.path.insert(0, "/opt/trn_rl_repo")

from contextlib import ExitStack

import numpy as np

import concourse.bacc as bacc
import concourse.mybir as mybir
import concourse.tile as tile
from concourse.bass_utils import run_bass_kernel_spmd
from concourse.masks import make_identity

B, S, D = 4, 8192, 1024
H, DH = 16, 64
SCALE = DH ** -0.5
NCORES = 8
S_LOC = S // NCORES          # 1024 rows per batch per core
ROWS = B * S_LOC             # 4096 rows per core
P = 128
TPB = S_LOC // P             # 8 tiles per batch element
NPAIR = H // 2               # 8 head pairs

F32 = mybir.dt.float32
F32R = mybir.dt.float32r
BF16 = mybir.dt.bfloat16
EXP = mybir.ActivationFunctionType.Exp
MUL = mybir.AluOpType.mult


def _load_weight(nc, pool, dram_ap, cols, tag, dtype=F32R):
    """Load a [D, cols] DRAM weight into SBUF as [128, D//128, cols]."""
    w = pool.tile([P, D // P, cols], dtype, tag=tag)
    src = dram_ap.rearrange("(c p) n -> p c n", p=P)
    if dtype == F32R:
        src = src.bitcast(F32R)
    nc.sync.dma_start(w[:], src)
    return w


def _transpose_128(nc, psum_tp, dst, src, ident):
    """PE-transpose a [128, 1024] tile into dst [128, 8, 128] (feature-major)."""
    for g in range(2):
        tp = psum_tp.tile([P, 512], F32, tag="tp")
        for k in range(4):
            c = g * 4 + k
            nc.tensor.transpose(tp[:, k * P:(k + 1) * P],
                                src[:, c * P:(c + 1) * P], ident)
        nc.vector.tensor_copy(out=dst[:, g * 4:(g + 1) * 4, :], in_=tp[:])


def _emit(tc, nc, x_d, wq_d, wkv_d, wlin_d, blin_d, y_d, reps, dbg=None):
    with ExitStack() as top:
        const = top.enter_context(tc.tile_pool(name="const", bufs=1))
        dram = top.enter_context(tc.tile_pool(name="dram", bufs=1, space="DRAM"))
        psum_mm = top.enter_context(tc.tile_pool(name="psum_mm", bufs=2, space="PSUM"))
        psum_tp = top.enter_context(tc.tile_pool(name="psum_tp", bufs=2, space="PSUM"))
        psum_acc = top.enter_context(tc.tile_pool(name="psum_acc", bufs=1, space="PSUM"))

        ident = const.tile([P, P], F32, tag="ident")
        make_identity(nc, ident)
        ident_bf = const.tile([P, P], BF16, tag="ident_bf")
        make_identity(nc, ident_bf)
        blin_bc = const.tile([P, D], F32, tag="blin_bc")
        nc.sync.dma_start(blin_bc[:], blin_d[None, :].to_broadcast([P, D]))

        for _ in range(reps):
            _emit_once(tc, nc, x_d, wq_d, wkv_d, wlin_d, y_d,
                       dram, psum_mm, psum_tp, psum_acc, ident, ident_bf, blin_bc,
                       dbg)


def _emit_once(tc, nc, x_d, wq_d, wkv_d, wlin_d, y_d,
               dram, psum_mm, psum_tp, psum_acc, ident, ident_bf, blin_bc,
               dbg=None):
    cc_in = dram.tile([P, B, NPAIR, 65], F32, tag="cc_in")
    cc_out = dram.tile([P, B, NPAIR, 65], F32, tag="cc_out")

    # ---------------- pass 1: KV -> ctxU/Z partials ----------------
    with ExitStack() as s1:
        p1 = s1.enter_context(tc.tile_pool(name="p1", bufs=2))
        p1c = s1.enter_context(tc.tile_pool(name="p1c", bufs=1))
        wkv_sb = _load_weight(nc, p1c, wkv_d, 2 * D, "wkv")
        ctx_loc = p1c.tile([P, B, NPAIR, 65], F32, tag="ctxloc")

        for b in range(B):
            # per-(b, head) accumulators: [d(2 heads on partitions), pair, e|Z]
            # NOTE: only one open accumulation group per PSUM bank is allowed
            # (start=True clears the whole bank's has_written), so each tile's
            # ctx matmuls are single-shot into a scratch bank and accumulated
            # into SBUF with a vector add.
            nc.vector.memset(ctx_loc[:, b], 0.0)
            for t in range(TPB):
                row0 = (b * TPB + t) * P
                x_nat = p1.tile([P, D], F32, tag="x")
                nc.sync.dma_start(x_nat[:], x_d[row0:row0 + P, :])
                xT = p1.tile([P, 8, P], F32R, tag="xT")
                _transpose_128(nc, psum_tp, xT, x_nat, ident)

                e_bf = p1.tile([P, H, DH], BF16, tag="E")
                v_bf = p1.tile([P, H, DH + 1], BF16, tag="V")
                for half in range(2):
                    kv_ps = psum_mm.tile([P, D], F32, tag="mm")
                    for nb in range(2):
                        col0 = half * D + nb * 512
                        for c in range(8):
                            nc.tensor.matmul(
                                kv_ps[:, nb * 512:(nb + 1) * 512],
                                lhsT=xT[:, c, :],
                                rhs=wkv_sb[:, c, col0:col0 + 512],
                                start=(c == 0), stop=(c == 7))
                    kv_v = kv_ps.rearrange("p (h t d) -> p h t d", t=2, d=DH)
                    hs = slice(half * 8, (half + 1) * 8)
                    nc.scalar.activation(e_bf[:, hs, :], kv_v[:, :, 0, :], EXP)
                    nc.vector.tensor_copy(out=v_bf[:, hs, 0:DH], in_=kv_v[:, :, 1, :])
                nc.vector.memset(v_bf[:, :, DH:DH + 1], 1.0)

                # per-pair stride padded to 128 floats so no matmul output
                # crosses a 2KB PSUM bank boundary
                acc = psum_acc.tile([P, NPAIR, P], F32, tag="acc")
                for j in range(NPAIR):
                    for odd in range(2):
                        h = 2 * j + odd
                        nc.tensor.matmul(
                            acc[odd * 64:(odd + 1) * 64, j, 0:65],
                            lhsT=e_bf[:, h, :],
                            rhs=v_bf[:, h, :],
                            start=True, stop=True,
                            tile_position=(0, odd * 64))
                nc.vector.tensor_add(out=ctx_loc[:, b], in0=ctx_loc[:, b],
                                     in1=acc[:, :, 0:65])

        nc.sync.dma_start(cc_in[:], ctx_loc[:])
        nc.gpsimd.collective_compute(
            "AllReduce", mybir.AluOpType.add,
            replica_groups=[list(range(NCORES))],
            ins=[cc_in.opt()], outs=[cc_out.opt()])

    # ---------------- mid: normalize ctx, build M = ctx @ Wlin ----------------
    span_cm = tc.tile_pool(name="span", bufs=1)
    span = span_cm.__enter__()
    try:
        wq_sb = _load_weight(nc, span, wq_d, D, "wq")
        m_sb = span.tile([P, B, 8, D], BF16, tag="M")

        with ExitStack() as s2:
            mid = s2.enter_context(tc.tile_pool(name="mid", bufs=1))
            midw = s2.enter_context(tc.tile_pool(name="midw", bufs=2))
            wlin_sb = _load_weight(nc, mid, wlin_d, D, "wlin")
            ctx_sb = mid.tile([P, B, NPAIR, 65], F32, tag="ctxg")
            nc.sync.dma_start(ctx_sb[:], cc_out[:])
            zinv = mid.tile([P, B, NPAIR], F32, tag="zinv")
            nc.vector.reciprocal(zinv[:], ctx_sb[:, :, :, 64])
            nc.vector.tensor_scalar_mul(zinv[:], zinv[:], SCALE)
            if dbg is not None:
                nc.sync.dma_start(dbg["ctx"], ctx_sb[:])
                nc.sync.dma_start(dbg["zinv"], zinv[:])

            for b in range(B):
                for j in range(NPAIR):
                    ctxn = midw.tile([P, P], F32, tag="ctxn")
                    nc.vector.memset(ctxn[:], 0.0)
                    for odd in range(2):
                        o = odd * 64
                        nc.vector.tensor_scalar_mul(
                            ctxn[o:o + 64, o:o + 64],
                            ctx_sb[o:o + 64, b, j, 0:64],
                            zinv[o:o + 64, b, j:j + 1])
                    tpp = psum_tp.tile([P, 512], F32, tag="tp")
                    nc.tensor.transpose(tpp[:, 0:P], ctxn[:], ident)
                    ctxnT = midw.tile([P, P], F32R, tag="ctxnT")
                    nc.vector.tensor_copy(out=ctxnT[:], in_=tpp[:, 0:P])
                    m_ps = psum_mm.tile([P, D], F32, tag="mm")
                    for nb in range(2):
                        nc.tensor.matmul(
                            m_ps[:, nb * 512:(nb + 1) * 512],
                            lhsT=ctxnT[:],
                            rhs=wlin_sb[:, j, nb * 512:(nb + 1) * 512],
                            start=True, stop=True)
                    nc.vector.tensor_copy(out=m_sb[:, b, j, :], in_=m_ps[:])

        if dbg is not None:
            nc.sync.dma_start(dbg["m"], m_sb[:])

        # ---------------- pass 2: Q -> y = Qn @ M + blin ----------------
        with ExitStack() as s3:
            p2 = s3.enter_context(tc.tile_pool(name="p2", bufs=2))
            for i in range(B * TPB):
                b = i // TPB
                row0 = i * P
                x_nat = p2.tile([P, D], F32, tag="x2")
                nc.sync.dma_start(x_nat[:], x_d[row0:row0 + P, :])
                xT = p2.tile([P, 8, P], F32R, tag="xT2")
                _transpose_128(nc, psum_tp, xT, x_nat, ident)

                q_ps = psum_mm.tile([P, D], F32, tag="mm")
                for nb in range(2):
                    for c in range(8):
                        nc.tensor.matmul(
                            q_ps[:, nb * 512:(nb + 1) * 512],
                            lhsT=xT[:, c, :],
                            rhs=wq_sb[:, c, nb * 512:(nb + 1) * 512],
                            start=(c == 0), stop=(c == 7))
                eq = p2.tile([P, H, DH], F32, tag="eq")
                nc.scalar.activation(eq[:], q_ps.rearrange("p (h d) -> p h d", d=DH), EXP)
                rs = p2.tile([P, H], F32, tag="rs")
                nc.vector.reduce_sum(rs[:], eq[:], axis=mybir.AxisListType.X)
                rsi = p2.tile([P, H], F32, tag="rsi")
                nc.vector.reciprocal(rsi[:], rs[:])
                qn = p2.tile([P, D], F32, tag="qn")
                nc.vector.tensor_tensor(
                    qn.rearrange("p (h d) -> p h d", d=DH),
                    eq[:],
                    rsi[:, :, None].to_broadcast([P, H, DH]),
                    MUL)
                qnT = p2.tile([P, 8, P], BF16, tag="qnT")
                for g in range(2):
                    tp2 = psum_tp.tile([P, 512], F32, tag="tp")
                    for k in range(4):
                        c = g * 4 + k
                        nc.tensor.transpose(tp2[:, k * P:(k + 1) * P],
                                            qn[:, c * P:(c + 1) * P], ident)
                    nc.vector.tensor_copy(out=qnT[:, g * 4:(g + 1) * 4, :], in_=tp2[:])

                y_ps = psum_mm.tile([P, D], F32, tag="mm")
                for nb in range(2):
                    for c in range(8):
                        nc.tensor.matmul(
                            y_ps[:, nb * 512:(nb + 1) * 512],
                            lhsT=qnT[:, c, :],
                            rhs=m_sb[:, b, c, nb * 512:(nb + 1) * 512],
                            start=(c == 0), stop=(c == 7))
                y_sb = p2.tile([P, D], F32, tag="y")
                nc.vector.tensor_add(out=y_sb[:], in0=y_ps[:], in1=blin_bc[:])
                nc.sync.dma_start(y_d[row0:row0 + P, :], y_sb[:])
                if dbg is not None and i == 0:
                    nc.sync.dma_start(dbg["eq"], eq[:])
                    nc.sync.dma_start(dbg["qnT"], qnT[:])
    finally:
        span_cm.__exit__(None, None, None)


_PROGRAM_CACHE = {}


def build_program(reps=1, debug_taps=False):
    key = (reps, debug_taps)
    if key in _PROGRAM_CACHE:
        return _PROGRAM_CACHE[key]
    nc = bacc.Bacc("TRN2", target_bir_lowering=False, debug=False,
                   num_devices=NCORES)
    x_d = nc.dram_tensor("x", [ROWS, D], F32, kind="ExternalInput").ap()
    wq_d = nc.dram_tensor("Wq", [D, D], F32, kind="ExternalInput").ap()
    wkv_d = nc.dram_tensor("Wkv", [D, 2 * D], F32, kind="ExternalInput").ap()
    wlin_d = nc.dram_tensor("Wlin", [D, D], F32, kind="ExternalInput").ap()
    blin_d = nc.dram_tensor("blin", [D], F32, kind="ExternalInput").ap()
    y_d = nc.dram_tensor("y", [ROWS, D], F32, kind="ExternalOutput").ap()
    dbg = None
    if debug_taps:
        dbg = {
            "ctx": nc.dram_tensor("dbg_ctx", [P, B, NPAIR, 65], F32,
                                  kind="ExternalOutput").ap(),
            "zinv": nc.dram_tensor("dbg_zinv", [P, B, NPAIR], F32,
                                   kind="ExternalOutput").ap(),
            "m": nc.dram_tensor("dbg_m", [P, B, 8, D], BF16,
                                kind="ExternalOutput").ap(),
            "eq": nc.dram_tensor("dbg_eq", [P, H, DH], F32,
                                 kind="ExternalOutput").ap(),
            "qnT": nc.dram_tensor("dbg_qnT", [P, 8, P], BF16,
                                  kind="ExternalOutput").ap(),
        }
    with tile.TileContext(nc) as tc:
        _emit(tc, nc, x_d, wq_d, wkv_d, wlin_d, blin_d, y_d, reps, dbg)
    nc.compile()
    _PROGRAM_CACHE[key] = nc
    return nc


def run_sharded(inputs, reps=1, debug_taps=False):
    """Run the SPMD program; returns (per-core result dicts)."""
    nc = build_program(reps, debug_taps)
    x = np.ascontiguousarray(inputs["x"], dtype=np.float32)
    wq = np.ascontiguousarray(inputs["Wq"], dtype=np.float32)
    wkv = np.ascontiguousarray(inputs["Wkv"], dtype=np.float32)
    wlin = np.ascontiguousarray(inputs["Wlin"], dtype=np.float32)
    blin = np.ascontiguousarray(inputs["blin"], dtype=np.float32)
    in_maps = []
    for c in range(NCORES):
        x_shard = np.ascontiguousarray(
            x[:, c * S_LOC:(c + 1) * S_LOC, :].reshape(ROWS, D))
        in_maps.append({"x": x_shard, "Wq": wq, "Wkv": wkv,
                        "Wlin": wlin, "blin": blin})
    res = run_bass_kernel_spmd(nc, in_maps, list(range(NCORES)))
    return res


def kernel(**inputs) -> np.ndarray:
    res = run_sharded(inputs, reps=1)
    shards = np.stack([res.results[c]["y"].reshape(B, S_LOC, D)
                       for c in range(NCORES)])          # [C, B, S_LOC, D]
    return np.ascontiguousarray(
        shards.transpose(1, 0, 2, 3).reshape(B, S, D))


if __name__ == "__main__":
    rng = np.random.default_rng(0)
    ins = {
        "x": rng.standard_normal((B, S, D), dtype=np.float32),
        "Wq": rng.standard_normal((D, D), dtype=np.float32) * 0.02,
        "Wkv": rng.standard_normal((D, 2 * D), dtype=np.float32) * 0.02,
        "Wlin": rng.standard_normal((D, D), dtype=np.float32) * 0.02,
        "blin": np.zeros(D, dtype=np.float32),
    }
    y = kernel(**ins)
    print("kernel output", y.shape, y.dtype, float(np.abs(y).mean()))


# revision 79
# speedup vs baseline: 249.5895x; 249.5895x over previous
"""Trainium2 Bass kernel for linear multi-head attention.

Reference computation (B=4, S=8192, D=1024, H=16, DH=64):
    Q  = softmax((x@Wq) per-head over DH) * DH**-0.5
    K  = softmax((x@Wkv)[...,:DH] per-head over S)
    V  = (x@Wkv)[..., DH:]
    ctx = K^T @ V  per (b, h)               # [DH, DH]
    y  = (Q @ ctx  per head) @ Wlin + blin

Sharding: sequence-parallel over 8 NeuronCores. Each core handles
S_LOC = 1024 rows per batch element. The K-softmax runs over the full
sequence, so each core accumulates unnormalized per-(b,h) context
ctxU = sum_s exp(k_s) v_s and Z = sum_s exp(k_s) locally, and a single
AllReduce sums them across cores. (No max-subtraction is needed:
|k| <= ~4 for these input statistics, exp stays in fp32 range.)

Per-core pipeline (all matmuls contract over the partition dim):
  pass 1: x tile -> PE-transpose -> KV = x@Wkv (fp32r, full rate)
          -> exp(K) (bf16) -> per-head ctxU/Z accumulation in PSUM
  allreduce [128, B*8*65] fp32 (ctxU + Z packed)
  mid:    ctxn = ctxU/Z * SCALE -> PE-transpose -> M_h = ctxn_h @ Wlin_h
          packed to M [D, D] per batch (bf16)  (y = Qn @ M + blin)
  pass 2: x tile -> PE-transpose -> Q = x@Wq (fp32r) -> exp
          -> row-normalize (free-dim reduce) -> PE-transpose (bf16)
          -> y = Qn @ M (bf16) + blin -> store
"""

import sys

if "/opt/trn_rl_repo" not in sys.path:
    sys.path.insert(0, "/opt/trn_rl_repo")

from contextlib import ExitStack

import numpy as np

import concourse.bacc as bacc
import concourse.mybir as mybir
import concourse.tile as tile
from concourse.bass_utils import run_bass_kernel_spmd
from concourse.masks import make_identity

B, S, D = 4, 8192, 1024
H, DH = 16, 64
SCALE = DH ** -0.5
NCORES = 8
S_LOC = S // NCORES          # 1024 rows per batch per core
ROWS = B * S_LOC             # 4096 rows per core
P = 128
TPB = S_LOC // P             # 8 tiles per batch element
NPAIR = H // 2               # 8 head pairs

F32 = mybir.dt.float32
F32R = mybir.dt.float32r
BF16 = mybir.dt.bfloat16
EXP = mybir.ActivationFunctionType.Exp
MUL = mybir.AluOpType.mult
PROJ_BF16 = True  # projection matmuls in bf16 instead of fp32r


def _load_weight(nc, pool, dram_ap, cols, tag, dtype=F32R, stg_pool=None):
    """Load a [D, cols] DRAM weight into SBUF as [128, D//128, cols]."""
    w = pool.tile([P, D // P, cols], dtype, tag=tag, name=f"w_{tag}")
    src = dram_ap.rearrange("(c p) n -> p c n", p=P)
    if dtype == F32R:
        nc.sync.dma_start(w[:], src.bitcast(F32R))
    elif dtype == BF16:
        stg = (stg_pool or pool).tile([P, D // P, cols], F32, tag="wstg",
                                      name=f"wstg_{tag}")
        nc.sync.dma_start(stg[:], src)
        nc.vector.tensor_copy(out=w[:], in_=stg[:])
    else:
        nc.sync.dma_start(w[:], src)
    return w


COPY = mybir.ActivationFunctionType.Copy


def _transpose_128(nc, psum_tp, dst, src, ident):
    """PE-transpose a [128, 1024] tile into dst [128, 8, 128] (feature-major).

    The PSUM->SBUF copies are split across DVE and ACT to balance engine load.
    """
    dt_ = src.dtype
    for g in range(2):
        tp = psum_tp.tile([P, 512], dt_, tag="tp")
        for k in range(4):
            c = g * 4 + k
            nc.tensor.transpose(tp[:, k * P:(k + 1) * P],
                                src[:, c * P:(c + 1) * P], ident)
        dst_v = dst[:, g * 4:(g + 1) * 4, :]
        if g == 0:
            nc.vector.tensor_copy(out=dst_v, in_=tp[:])
        else:
            nc.scalar.activation(dst_v, tp[:], COPY)


def _emit(tc, nc, x_d, wq_d, wkv_d, wlin_d, blin_d, y_d, reps, dbg=None,
          no_collective=False):
    with ExitStack() as top:
        const = top.enter_context(tc.tile_pool(name="const", bufs=1))
        dram = top.enter_context(tc.tile_pool(name="dram", bufs=1, space="DRAM"))
        # One set of top-level PSUM pools shared by every phase; keeping them
        # alive for the whole kernel avoids released-zone dependencies that
        # would chain pass-2 matmuls behind the collective.
        #   psum_mm [128,1024] x2 (4 banks): KV halves, M-build, y
        #   psum_aq [128,1024] x1 (2 banks): pass-1 ctx scratch + pass-2 Q
        #   psum_tp [128,512]  x2 (2 banks): all PE transposes
        psum_mm = top.enter_context(tc.tile_pool(name="psum_mm", bufs=2, space="PSUM"))
        psum_aq = top.enter_context(tc.tile_pool(name="psum_aq", bufs=1, space="PSUM"))
        psum_tp = top.enter_context(tc.tile_pool(name="psum_tp", bufs=2, space="PSUM"))

        ident = const.tile([P, P], F32, tag="ident")
        make_identity(nc, ident)
        ident_bf = const.tile([P, P], BF16, tag="ident_bf")
        make_identity(nc, ident_bf)
        blin_bc = const.tile([P, D], F32, tag="blin_bc")
        nc.sync.dma_start(blin_bc[:], blin_d[None, :].to_broadcast([P, D]))

        psum = (psum_mm, psum_aq, psum_tp)
        for _ in range(reps):
            _emit_once(tc, nc, x_d, wq_d, wkv_d, wlin_d, y_d,
                       dram, psum, (ident, ident_bf), blin_bc, dbg,
                       no_collective)


def _emit_once(tc, nc, x_d, wq_d, wkv_d, wlin_d, y_d,
               dram, psum, idents, blin_bc, dbg=None, no_collective=False):
    psum_mm, psum_aq, psum_tp = psum
    ident, ident_bf = idents
    cc_in = dram.tile([P, B, NPAIR, 65], F32, tag="cc_in")
    cc_out = dram.tile([P, B, NPAIR, 65], F32, tag="cc_out",
                       addr_space="Shared")

    # Wq is prefetched up front (its own pool, disjoint SBUF) so the pass-2
    # Q matmuls can start during the collective.
    wq_cm = tc.tile_pool(name="wqp", bufs=1)
    wqp = wq_cm.__enter__()

    def load_xT(pool, row0, tag_x, tag_xT, bufs_xT=3):
        x_nat = pool.tile([P, D], F32, tag=tag_x, name=f"x_{tag_x}", bufs=2)
        nc.sync.dma_start(x_nat[:], x_d[row0:row0 + P, :])
        if PROJ_BF16:
            # cast before the transposes so they run at bf16 rate on the PE
            xbf = pool.tile([P, D], BF16, tag=f"{tag_x}b", name=f"xb_{tag_x}",
                            bufs=2)
            nc.scalar.activation(xbf[:], x_nat[:], COPY)
            src_t, idt = xbf, ident_bf
        else:
            src_t, idt = x_nat, ident
        xT = pool.tile([P, 8, P], BF16 if PROJ_BF16 else F32R,
                       tag=tag_xT, name=f"xT_{tag_xT}", bufs=bufs_xT)
        _transpose_128(nc, psum_tp, xT, src_t, idt)
        return xT

    # ---------------- pass 1: KV -> ctxU/Z partials ----------------
    with ExitStack() as s1:
        p1 = s1.enter_context(tc.tile_pool(name="p1", bufs=2))
        p1c = s1.enter_context(tc.tile_pool(name="p1c", bufs=1))

        # first tiles' load+transpose emitted interleaved with the big weight
        # DMAs so the PE isn't idle behind them at kernel start (the DMA
        # queue drains in emission order)
        xT_pre = [load_xT(p1, 0, "x", "xT", bufs_xT=7)]
        wdt = BF16 if PROJ_BF16 else F32R
        wkv_sb = p1c.tile([P, D // P, 2 * D], wdt, tag="wkv")
        wkv_src = wkv_d.rearrange("(c p) n -> p c n", p=P)
        if not PROJ_BF16:
            wkv_src = wkv_src.bitcast(F32R)
            for ci, t in ((0, 1), (2, 2), (4, 3), (6, None)):
                nc.sync.dma_start(wkv_sb[:, ci:ci + 2, :],
                                  wkv_src[:, ci:ci + 2, :])
                if t is not None:
                    xT_pre.append(load_xT(p1, t * P, "x", "xT", bufs_xT=7))
        else:
            for ci in range(8):
                wst = p1.tile([P, 1, 2 * D], F32, tag="wst", name="wst", bufs=2)
                nc.sync.dma_start(wst[:], wkv_src[:, ci:ci + 1, :])
                nc.vector.tensor_copy(out=wkv_sb[:, ci:ci + 1, :], in_=wst[:])
                if ci >= 2 and len(xT_pre) < 6:
                    xT_pre.append(load_xT(p1, len(xT_pre) * P, "x", "xT",
                                          bufs_xT=7))
        wq_sb = _load_weight(nc, wqp, wq_d, D, "wq",
                             dtype=BF16 if PROJ_BF16 else F32R, stg_pool=p1)
        ctx_loc = wqp.tile([P, B, NPAIR, 65], F32, tag="ctxloc")

        for b in range(B):
            # per-(b, head) accumulators: [d(2 heads on partitions), pair, e|Z]
            # NOTE: only one open accumulation group per PSUM bank is allowed
            # (start=True clears the whole bank's has_written), so each tile's
            # ctx matmuls are single-shot into a scratch bank and accumulated
            # into SBUF with a vector add.
            nc.vector.memset(ctx_loc[:, b], 0.0)
            for t in range(TPB):
                row0 = (b * TPB + t) * P
                if b == 0 and t < len(xT_pre):
                    xT = xT_pre[t]
                else:
                    xT = load_xT(p1, row0, "x", "xT", bufs_xT=7)

                e_bf = p1.tile([P, H, DH], BF16, tag="E")
                v_bf = p1.tile([P, H, DH + 1], BF16, tag="V")
                for half in range(2):
                    kv_ps = psum_mm.tile([P, D], F32, tag="mm", name="kv_ps")
                    for nb in range(2):
                        col0 = half * D + nb * 512
                        for c in range(8):
                            nc.tensor.matmul(
                                kv_ps[:, nb * 512:(nb + 1) * 512],
                                lhsT=xT[:, c, :],
                                rhs=wkv_sb[:, c, col0:col0 + 512],
                                start=(c == 0), stop=(c == 7))
                    kv_v = kv_ps.rearrange("p (h t d) -> p h t d", t=2, d=DH)
                    hs = slice(half * 8, (half + 1) * 8)
                    nc.scalar.activation(e_bf[:, hs, :], kv_v[:, :, 0, :], EXP)
                    nc.scalar.activation(v_bf[:, hs, 0:DH], kv_v[:, :, 1, :], COPY)
                nc.vector.memset(v_bf[:, :, DH:DH + 1], 1.0)

                # per-pair stride padded to 128 floats so no matmul output
                # crosses a 2KB PSUM bank boundary
                acc_t = psum_aq.tile([P, D], F32, tag="aq", name="acc")
                acc = acc_t.rearrange("p (j k) -> p j k", k=P)
                for j in range(NPAIR):
                    for odd in range(2):
                        h = 2 * j + odd
                        nc.tensor.matmul(
                            acc[odd * 64:(odd + 1) * 64, j, 0:65],
                            lhsT=e_bf[:, h, :],
                            rhs=v_bf[:, h, :],
                            start=True, stop=True,
                            tile_position=(0, odd * 64))
                nc.vector.tensor_add(out=ctx_loc[:, b], in0=ctx_loc[:, b],
                                     in1=acc[:, :, 0:65])
            if b == 0:
                # load Wlin while the DMA queue is otherwise quiet, so the
                # M-build isn't waiting on it at the pass boundary
                wlin_sb = _load_weight(nc, wqp, wlin_d, D, "wlin",
                                       stg_pool=p1)

    # ---------------- pass 2 + mid, overlapped ----------------
    HOIST = 6  # stage-A tiles emitted before the M-build to cover the
    #            collective + M-build latency with PE work
    span_cm = tc.tile_pool(name="span", bufs=1)
    span = span_cm.__enter__()
    p2_cm = tc.tile_pool(name="p2", bufs=3)
    p2 = p2_cm.__enter__()
    try:
        m_sb = span.tile([P, B, 8, D], BF16, tag="M")

        def q_stage_a(i):
            row0 = i * P
            xT = load_xT(p2, row0, "x2", "xT2")
            q_ps = psum_aq.tile([P, D], F32, tag="aq", name="q_ps")
            for nb in range(2):
                for c in range(8):
                    nc.tensor.matmul(
                        q_ps[:, nb * 512:(nb + 1) * 512],
                        lhsT=xT[:, c, :],
                        rhs=wq_sb[:, c, nb * 512:(nb + 1) * 512],
                        start=(c == 0), stop=(c == 7))
            eq = p2.tile([P, H, DH], BF16, tag="eq", name="eq",
                         bufs=HOIST + 2)
            nc.scalar.activation(
                eq[:], q_ps.rearrange("p (h d) -> p h d", d=DH), EXP)
            return eq

        def q_stage_b(i, eq):
            b = i // TPB
            row0 = i * P
            rs = p2.tile([P, H], F32, tag="rs", name="rs")
            nc.vector.reduce_sum(rs[:], eq[:], axis=mybir.AxisListType.X)
            rsi = p2.tile([P, H], F32, tag="rsi", name="rsi")
            nc.vector.reciprocal_approx_fast(rsi[:], rs[:])
            rsb = p2.tile([P, H], BF16, tag="rsb", name="rsb")
            nc.vector.tensor_copy(out=rsb[:], in_=rsi[:])
            qn = p2.tile([P, D], BF16, tag="qn", name="qn")
            nc.vector.tensor_tensor(
                qn.rearrange("p (h d) -> p h d", d=DH),
                eq[:],
                rsb[:, :, None].to_broadcast([P, H, DH]),
                MUL)
            qnT = p2.tile([P, 8, P], BF16, tag="qnT", name="qnT")
            for g in range(2):
                tp2 = psum_tp.tile([P, 512], BF16, tag="tp", name="tp2")
                for k in range(4):
                    c = g * 4 + k
                    nc.tensor.transpose(tp2[:, k * P:(k + 1) * P],
                                        qn[:, c * P:(c + 1) * P], ident_bf)
                nc.vector.tensor_copy(out=qnT[:, g * 4:(g + 1) * 4, :],
                                      in_=tp2[:])

            y_ps = psum_mm.tile([P, D], F32, tag="mm", name="y_ps")
            for nb in range(2):
                for c in range(8):
                    nc.tensor.matmul(
                        y_ps[:, nb * 512:(nb + 1) * 512],
                        lhsT=qnT[:, c, :],
                        rhs=m_sb[:, b, c, nb * 512:(nb + 1) * 512],
                        start=(c == 0), stop=(c == 7))
            y_sb = p2.tile([P, D], F32, tag="ysb", name="y_sb")
            nc.vector.tensor_add(out=y_sb[:], in0=y_ps[:], in1=blin_bc[:])
            nc.sync.dma_start(y_d[row0:row0 + P, :], y_sb[:])
            if dbg is not None and i == 0:
                nc.sync.dma_start(dbg["eq"], eq[:])
                nc.sync.dma_start(dbg["qnT"], qnT[:])

        from collections import deque
        pend = deque()
        for i in range(2):
            pend.append((i, q_stage_a(i)))

        # collective emitted after the first pass-2 loads so the DMA queue
        # feeds the PE before the bounce buffers
        nc.sync.dma_start(cc_in[:], ctx_loc[:])
        if no_collective:
            nc.sync.dma_start(cc_out[:], cc_in[:])
        else:
            nc.gpsimd.collective_compute(
                "AllReduce", mybir.AluOpType.add,
                replica_groups=[list(range(NCORES))],
                ins=[cc_in.opt()], outs=[cc_out.opt()])

        for i in range(2, HOIST):
            pend.append((i, q_stage_a(i)))

        # ---- mid: normalize ctx, build M = ctx @ Wlin (pipelined) ----
        with ExitStack() as s2:
            mid = s2.enter_context(tc.tile_pool(name="mid", bufs=1))
            midw = s2.enter_context(tc.tile_pool(name="midw", bufs=4))
            ctx_sb = mid.tile([P, B, NPAIR, 65], F32, tag="ctxg")
            nc.sync.dma_start(ctx_sb[:], cc_out[:])
            zinv = mid.tile([P, B, NPAIR], F32, tag="zinv")
            nc.vector.reciprocal_approx_fast(zinv[:], ctx_sb[:, :, :, 64])
            nc.vector.tensor_scalar_mul(zinv[:], zinv[:], SCALE)
            if dbg is not None:
                nc.sync.dma_start(dbg["ctx"], ctx_sb[:])
                nc.sync.dma_start(dbg["zinv"], zinv[:])

            def m_stage_a(b, j):
                ctxn = midw.tile([P, P], F32, tag="ctxn", name="ctxn")
                nc.vector.memset(ctxn[:], 0.0)
                for odd in range(2):
                    o = odd * 64
                    nc.vector.tensor_scalar_mul(
                        ctxn[o:o + 64, o:o + 64],
                        ctx_sb[o:o + 64, b, j, 0:64],
                        zinv[o:o + 64, b, j:j + 1])
                tpp = psum_tp.tile([P, 512], F32, tag="tp", name="tpp")
                nc.tensor.transpose(tpp[:, 0:P], ctxn[:], ident)
                return tpp

            def m_stage_b(b, j, tpp):
                ctxnT = midw.tile([P, P], F32R, tag="ctxnT", name="ctxnT")
                nc.vector.tensor_copy(out=ctxnT[:], in_=tpp[:, 0:P])
                m_ps = psum_mm.tile([P, D], F32, tag="mm", name="m_ps")
                for nb in range(2):
                    nc.tensor.matmul(
                        m_ps[:, nb * 512:(nb + 1) * 512],
                        lhsT=ctxnT[:],
                        rhs=wlin_sb[:, j, nb * 512:(nb + 1) * 512],
                        start=True, stop=True)
                nc.scalar.activation(m_sb[:, b, j, :], m_ps[:], COPY)

            mpend = None
            for b in range(B):
                for j in range(NPAIR):
                    tpp = m_stage_a(b, j)
                    if mpend is not None:
                        m_stage_b(*mpend)
                    mpend = (b, j, tpp)
            m_stage_b(*mpend)

        if dbg is not None:
            nc.sync.dma_start(dbg["m"], m_sb[:])

        # ---- pass 2 steady state ----
        for i in range(HOIST, B * TPB):
            pend.append((i, q_stage_a(i)))
            q_stage_b(*pend.popleft())
        while pend:
            q_stage_b(*pend.popleft())
    finally:
        p2_cm.__exit__(None, None, None)
        span_cm.__exit__(None, None, None)
        wq_cm.__exit__(None, None, None)


_PROGRAM_CACHE = {}


def build_program(reps=1, debug_taps=False, single_core=False):
    key = (reps, debug_taps, single_core, PROJ_BF16)
    if key in _PROGRAM_CACHE:
        return _PROGRAM_CACHE[key]
    nc = bacc.Bacc("TRN2", target_bir_lowering=False, debug=False,
                   num_devices=1 if single_core else NCORES)
    x_d = nc.dram_tensor("x", [ROWS, D], F32, kind="ExternalInput").ap()
    wq_d = nc.dram_tensor("Wq", [D, D], F32, kind="ExternalInput").ap()
    wkv_d = nc.dram_tensor("Wkv", [D, 2 * D], F32, kind="ExternalInput").ap()
    wlin_d = nc.dram_tensor("Wlin", [D, D], F32, kind="ExternalInput").ap()
    blin_d = nc.dram_tensor("blin", [D], F32, kind="ExternalInput").ap()
    y_d = nc.dram_tensor("y", [ROWS, D], F32, kind="ExternalOutput").ap()
    dbg = None
    if debug_taps:
        dbg = {
            "ctx": nc.dram_tensor("dbg_ctx", [P, B, NPAIR, 65], F32,
                                  kind="ExternalOutput").ap(),
            "zinv": nc.dram_tensor("dbg_zinv", [P, B, NPAIR], F32,
                                   kind="ExternalOutput").ap(),
            "m": nc.dram_tensor("dbg_m", [P, B, 8, D], BF16,
                                kind="ExternalOutput").ap(),
            "eq": nc.dram_tensor("dbg_eq", [P, H, DH], BF16,
                                 kind="ExternalOutput").ap(),
            "qnT": nc.dram_tensor("dbg_qnT", [P, 8, P], BF16,
                                  kind="ExternalOutput").ap(),
        }
    with tile.TileContext(nc) as tc:
        _emit(tc, nc, x_d, wq_d, wkv_d, wlin_d, blin_d, y_d, reps, dbg,
              no_collective=single_core)
    nc.compile()
    _PROGRAM_CACHE[key] = nc
    return nc


def run_sharded(inputs, reps=1, debug_taps=False):
    """Run the SPMD program; returns (per-core result dicts)."""
    nc = build_program(reps, debug_taps)
    x = np.ascontiguousarray(inputs["x"], dtype=np.float32)
    wq = np.ascontiguousarray(inputs["Wq"], dtype=np.float32)
    wkv = np.ascontiguousarray(inputs["Wkv"], dtype=np.float32)
    wlin = np.ascontiguousarray(inputs["Wlin"], dtype=np.float32)
    blin = np.ascontiguousarray(inputs["blin"], dtype=np.float32)
    in_maps = []
    for c in range(NCORES):
        x_shard = np.ascontiguousarray(
            x[:, c * S_LOC:(c + 1) * S_LOC, :].reshape(ROWS, D))
        in_maps.append({"x": x_shard, "Wq": wq, "Wkv": wkv,
                        "Wlin": wlin, "blin": blin})
    res = run_bass_kernel_spmd(nc, in_maps, list(range(NCORES)))
    return res


def kernel(**inputs) -> np.ndarray:
    res = run_sharded(inputs, reps=1)
    shards = np.stack([res.results[c]["y"].reshape(B, S_LOC, D)
                       for c in range(NCORES)])          # [C, B, S_LOC, D]
    return np.ascontiguousarray(
        shards.transpose(1, 0, 2, 3).reshape(B, S, D))


if __name__ == "__main__":
    rng = np.random.default_rng(0)
    ins = {
        "x": rng.standard_normal((B, S, D), dtype=np.float32),
        "Wq": rng.standard_normal((D, D), dtype=np.float32) * 0.02,
        "Wkv": rng.standard_normal((D, 2 * D), dtype=np.float32) * 0.02,
        "Wlin": rng.standard_normal((D, D), dtype=np.float32) * 0.02,
        "blin": np.zeros(D, dtype=np.float32),
    }
    y = kernel(**ins)
    print("kernel output", y.shape, y.dtype, float(np.abs(y).mean()))
